# revision 23
# baseline (speedup 1.0000x reference)
"""Trainium2 Bass kernel for nn_Bert_44452911514066 (DeBERTa-style disentangled
attention BERT layer), data-parallel over batch across 8 NeuronCores.

kernel(**inputs) takes the FULL inputs (as produced by reference.setup_inputs)
and returns the FULL [S, B, H] output.

Key ideas (v2):
  - batch-DP: 2 batches per core, weights/tables replicated.
  - relative-position gather is Toeplitz: per (b,h) bucket values expand into
    diagonal space via matmuls (M matrices built from one-hot expansions),
    then sheared via single SBUF->SBUF DMAs with diagonal access patterns.
  - scores assembled transposed [k, q] in PSUM: CC matmul (+row-paired heads)
    + identity-matmul of (transposed cq + sheared ck), with the cq transpose
    done in f16 via PE transpose into an f16 PSUM tile.
  - softmax without max-subtraction: exp(s - 12); mask + denominator folded
    into an augmented/masked V matrix; division on DVE.
  - all matmuls f16 (full PE rate), fp32 accumulation.
  - on-chip PE transposes for h/ln2 (no DRAM round trip); expansion + CC
    matmuls of even/odd heads issued adjacently on disjoint PE row groups.
"""
import sys
sys.path.insert(0, "/opt/trn_rl_repo")
import math
import functools
import contextlib
import numpy as np

import concourse.bass as bass
import concourse.tile as tile
from concourse import mybir
from concourse.masks import make_identity

H, NH, HD, S, B = 768, 12, 64, 512, 16
NCORES = 8
BL = B // NCORES          # batches per core
T = BL * S                # tokens per core
SCALE = 1.0 / math.sqrt(3 * HD)
EPS = 1e-7
NB = 63                   # relative buckets
WIN = 640                 # expansion window per 128-row tile
CSHIFT = 12.0             # exp shift
F16 = mybir.dt.float16
F32 = mybir.dt.float32
AF = mybir.ActivationFunctionType
OP = mybir.AluOpType

# ---------------------------------------------------------------------------
# walrus workaround: this container's walrus accepts at most ONE sync wait per
# instruction; split extra waits onto single-wait NoOps.
# ---------------------------------------------------------------------------
from concourse.vector_clock import ScopedClock

_orig_add_instruction = tile.TileContext._add_instruction


def _patched_add_instruction(self, inst):
    si = inst.sync_info
    if si is not None and si.on_wait is not None and len(si.on_wait) > 1:
        waits = list(si.on_wait)
        for i, w in enumerate(waits[:-1]):
            nop = mybir.InstNoOp(name=f"{inst.name}-wsplit{i}", ins=[], outs=[])
            nop.engine = inst.engine
            nop.sync_info = mybir.SyncInfo(on_wait=[w], on_update=[])
            _orig_add_instruction(self, nop)
        inst.sync_info = mybir.SyncInfo(
            on_wait=[waits[-1]], on_update=list(si.on_update or []))
    _orig_add_instruction(self, inst)


def _patched_drain_and_barrier(self, tick_clock, wait_clock):
    nc = self.nc
    probe = nc.sync.nop(nofuse=True)
    wait_clock.add_sem_waits(probe.ins, ScopedClock({None: tick_clock.global_clock}))
    si = probe.ins.sync_info
    waits = list(si.on_wait) if si is not None and si.on_wait else []
    if len(waits) > 1:
        probe.ins.sync_info = mybir.SyncInfo(on_wait=waits[:1], on_update=[])
        for w in waits[1:]:
            n2 = nc.sync.nop(nofuse=True)
            n2.ins.sync_info = mybir.SyncInfo(on_wait=[w], on_update=[])
    nc.sync.drain()
    nc.all_engine_barrier()
    assert self.sems is not None
    popped = nc._tile_sem_poison_stack.pop()
    assert popped is self._sem_poison
    nc.clear_and_free_semaphores(list(self.sems.allocated().values()))
    nc.all_engine_barrier()


tile.TileContext._add_instruction = _patched_add_instruction
tile.TileContext._drain_and_barrier = _patched_drain_and_barrier


def _shear4_ap(t_ap):
    """Batched shear over 4 stacked windows.

    t_ap is a [128, 4, WIN] tile (partition p, window t, diag j).
    out[p, t, k] = tile_flat[p*(4*WIN-1) + t*WIN + 127 + k]
                 = tile[p, t, 127 + k - p]   for k in [0, 512).
    """
    return bass.AP(tensor=t_ap.tensor, offset=t_ap.offset + 127,
                   ap=[[4 * WIN - 1, 128], [WIN, 4], [1, 512]])


# ---------------------------------------------------------------------------
# device kernel build
# ---------------------------------------------------------------------------
@functools.lru_cache(maxsize=2)
def build_module(with_bias: bool):
    nc = bass.Bass()

    hid_d = nc.dram_tensor("hid", [T, H], F32, kind="ExternalInput")
    wqkT_d = nc.dram_tensor("wqkT", [H, 2 * H], F16, kind="ExternalInput")
    wvgT_d = nc.dram_tensor("wvgT", [H, 2 * H], F16, kind="ExternalInput")
    woutT_d = nc.dram_tensor("woutT", [H, H], F16, kind="ExternalInput")
    relT_d = nc.dram_tensor("relT", [H, NB], F16, kind="ExternalInput")
    Ecq_d = nc.dram_tensor("Ecq", [NB, 1024], F16, kind="ExternalInput")
    Eck_d = nc.dram_tensor("Eck", [NB, 1024], F16, kind="ExternalInput")
    vmask_d = nc.dram_tensor("vmask", [T, 1], F32, kind="ExternalInput")
    if with_bias:
        # host-prepared: bqkc[p, f] = b_qk[128f+p] * (SCALE if f<6 else 1)
        bqkc_d = nc.dram_tensor("bqkc", [128, 12], F32, kind="ExternalInput")
        # rows replicated for free-dim adds
        bqkr_d = nc.dram_tensor("bqkr", [1, 2 * H], F32, kind="ExternalInput")
        bvgr_d = nc.dram_tensor("bvgr", [1, 2 * H], F32, kind="ExternalInput")
        boutr_d = nc.dram_tensor("boutr", [1, H], F32, kind="ExternalInput")
    out_d = nc.dram_tensor("out", [T, H], F32, kind="ExternalOutput")

    with tile.TileContext(nc) as tc, contextlib.ExitStack() as ctx:
        persist = ctx.enter_context(tc.tile_pool(name="persist", bufs=1))
        stats = ctx.enter_context(tc.tile_pool(name="stats", bufs=4))
        # pools that live only through phases 1-2 (freed before attention)
        ph2stack = contextlib.ExitStack()
        ph2sb = ph2stack.enter_context(tc.tile_pool(name="ph2sb", bufs=1))

        # --- constants ---
        ident16 = persist.tile([128, 128], F16, tag="id16")
        make_identity(nc, ident16)
        eps_t = persist.tile([128, 1], F32, tag="eps")
        nc.vector.memset(eps_t, EPS)
        negc_t = persist.tile([128, 1], F32, tag="negc")
        nc.vector.memset(negc_t, -CSHIFT)

        # --- load weights / tables ---
        # hid tiles + vmask go first on the sync HWDGE ring (critical path for
        # LN1); all weight tables ride the scalar HWDGE ring.
        vmask16 = persist.tile([128, 8], F32, tag="vm")
        nc.sync.dma_start(
            out=vmask16[:],
            in_=vmask_d[:].rearrange("(t p) one -> p (t one)", p=128))
        wqkT = ph2sb.tile([128, 6, 2 * H], F16, tag="wqkT")
        wvgT = ph2sb.tile([128, 6, 2 * H], F16, tag="wvgT")
        woutT = persist.tile([128, 6, H], F16, tag="woutT")
        relT = ph2sb.tile([128, 6, NB], F16, tag="relT")
        # early tables on the scalar HWDGE ring; late ones on the gpsimd
        # SWDGE ring; hid tiles keep the sync ring to themselves.
        for c in range(6):
            nc.scalar.dma_start(out=relT[:, c, :], in_=relT_d[128 * c:128 * c + 128, :])
        for c in range(6):
            nc.scalar.dma_start(out=wqkT[:, c, :], in_=wqkT_d[128 * c:128 * c + 128, :])
        Ecq = ph2sb.tile([NB, 1024], F16, tag="Ecq")
        Eck = ph2sb.tile([NB, 1024], F16, tag="Eck")
        nc.scalar.dma_start(out=Ecq[:], in_=Ecq_d[:])
        nc.scalar.dma_start(out=Eck[:], in_=Eck_d[:])
        for c in range(6):
            nc.gpsimd.dma_start(out=wvgT[:, c, :], in_=wvgT_d[128 * c:128 * c + 128, :])
        for c in range(6):
            nc.gpsimd.dma_start(out=woutT[:, c, :], in_=woutT_d[128 * c:128 * c + 128, :])
        if with_bias:
            bqkc = ph2sb.tile([128, 12], F32, tag="bqkc")
            nc.scalar.dma_start(out=bqkc[:], in_=bqkc_d[:])
            bqkr = ph2sb.tile([64, 2 * H], F32, tag="bqkr")
            nc.scalar.dma_start(
                out=bqkr[:],
                in_=bass.AP(tensor=bqkr_d, offset=0, ap=[[0, 64], [1, 2 * H]]))
            bvgr = ph2sb.tile([128, 2 * H], F32, tag="bvgr")
            nc.scalar.dma_start(
                out=bvgr[:],
                in_=bass.AP(tensor=bvgr_d, offset=0, ap=[[0, 128], [1, 2 * H]]))
            boutr = persist.tile([128, H], F32, tag="boutr")
            nc.scalar.dma_start(
                out=boutr[:],
                in_=bass.AP(tensor=boutr_d, offset=0, ap=[[0, 128], [1, H]]))

        def layernorm_to(out16, xin, tag):
            st = stats.tile([128, 3, 6], F32, tag="bnst")
            for sg in range(3):
                nc.vector.bn_stats(out=st[:, sg, :], in_=xin[:, 256 * sg:256 * sg + 256])
            mv = stats.tile([128, 2], F32, tag="bnmv")
            nc.vector.bn_aggr(out=mv[:], in_=st[:])
            rstd = stats.tile([128, 1], F32, tag="rstd")
            nc.scalar.activation(out=rstd[:], in_=mv[:, 1:2], func=AF.Sqrt,
                                 bias=eps_t[:], scale=1.0)
            nc.vector.reciprocal(out=rstd[:], in_=rstd[:])
            nc.vector.scalar_tensor_tensor(
                out=out16, in0=xin, scalar=mv[:, 0:1],
                in1=rstd[:].to_broadcast((128, H)),
                op0=OP.subtract, op1=OP.mult)

        # persistent activations
        hT = ph2sb.tile([128, 6, T], F16, tag="hT")
        qk16 = persist.tile([128, 12, T], F16, tag="qk16")
        g16 = persist.tile([128, 8, H], F16, tag="g16")
        va16 = persist.tile([128, 8, NH * 65], F16, tag="va16")
        ctx16a = persist.tile([128, 4, H], F16, tag="ctx16a")
        ctx16b = persist.tile([128, 4, H], F16, tag="ctx16b")
        posp = ph2sb.tile([64, 2 * H], F16, tag="posp")
        Mh = persist.tile([128, 6, 1024], F16, tag="Mh")
        Mq = persist.tile([128, 6, 1024], F16, tag="Mq")

        # =================================================================
        # phase 1+2: pos/M build, LN1 + on-chip transpose, QK/VG projections
        # =================================================================
        if True:
            ph2ps = ph2stack.enter_context(
                tc.tile_pool(name="ph2ps", bufs=4, space="PSUM"))
            tps = ph2stack.enter_context(
                tc.tile_pool(name="tps", bufs=4, space="PSUM"))
            ph2 = ph2stack.enter_context(tc.tile_pool(name="ph2", bufs=3))
            # --- LN1 + PE transpose into hT (first: hid DMAs land quickly) ---
            for t in range(8):
                xt = ph2.tile([128, H], F32, tag="x")
                nc.sync.dma_start(out=xt[:], in_=hid_d[128 * t:128 * t + 128, :])
                h16 = ph2.tile([128, H], F16, tag="h16")
                layernorm_to(h16[:], xt[:], f"ln1_{t}")
                for c in range(6):
                    tp = tps.tile([128, 128], F16, tag="tp")
                    nc.tensor.matmul(tp[:], h16[:, 128 * c:128 * c + 128],
                                     ident16[:], is_transpose=True,
                                     start=True, stop=True)
                    if c % 2 == 0:
                        nc.scalar.activation(
                            out=hT[:, c, 128 * t:128 * t + 128], in_=tp[:],
                            func=AF.Copy)
                    else:
                        nc.vector.tensor_copy(
                            out=hT[:, c, 128 * t:128 * t + 128], in_=tp[:])
            # --- pos projection ---
            for fc in range(3):
                ps = ph2ps.tile([128, 512], F32, tag="ps2")
                for c in range(6):
                    nc.tensor.matmul(
                        ps[:NB, :], relT[:, c, :], wqkT[:, c, 512 * fc:512 * fc + 512],
                        start=(c == 0), stop=(c == 5))
                if fc == 0:
                    segs = [(0, 512, SCALE)]
                elif fc == 1:
                    segs = [(0, 256, SCALE), (256, 512, 1.0)]
                else:
                    segs = [(0, 512, 1.0)]
                for (a, b_, sc) in segs:
                    if with_bias:
                        nc.vector.scalar_tensor_tensor(
                            out=posp[:NB, 512 * fc + a:512 * fc + b_],
                            in0=ps[:NB, a:b_], scalar=float(sc),
                            in1=bqkr[:NB, 512 * fc + a:512 * fc + b_],
                            op0=OP.mult, op1=OP.add)
                    else:
                        nc.vector.tensor_scalar_mul(
                            out=posp[:NB, 512 * fc + a:512 * fc + b_],
                            in0=ps[:NB, a:b_], scalar1=float(sc))
            # --- M matrices (per head pair; odd head in partitions 64-127;
            #     the two halves issued adjacently -> concurrent col groups) ---
            for p in range(6):
                for ec in range(2):
                    psq = ph2ps.tile([128, 512], F32, tag="ps2")
                    psk = ph2ps.tile([128, 512], F32, tag="ps2")
                    for half in range(2):
                        hh = 2 * p + half
                        r0 = 64 * half
                        nc.tensor.matmul(
                            psq[r0:r0 + 64, :],
                            posp[:NB, H + 64 * hh:H + 64 * hh + 64],
                            Ecq[:, 512 * ec:512 * ec + 512],
                            start=True, stop=True, tile_position=(0, r0))
                    for half in range(2):
                        hh = 2 * p + half
                        r0 = 64 * half
                        nc.tensor.matmul(
                            psk[r0:r0 + 64, :],
                            posp[:NB, 64 * hh:64 * hh + 64],
                            Eck[:, 512 * ec:512 * ec + 512],
                            start=True, stop=True, tile_position=(0, r0))
                    nc.scalar.activation(
                        out=Mh[:, p, 512 * ec:512 * ec + 512],
                        in_=psq[:], func=AF.Copy)
                    nc.vector.tensor_copy(
                        out=Mq[:, p, 512 * ec:512 * ec + 512],
                        in_=psk[:])

            # --- QK (feature-major; order so attention deps finish early) ---
            for p in range(6):
                for f in (p, 6 + p):
                    for nh in range(2):
                        ps = ph2ps.tile([128, 512], F32, tag="ps2")
                        for c in range(6):
                            nc.tensor.matmul(
                                ps[:], wqkT[:, c, 128 * f:128 * f + 128],
                                hT[:, c, 512 * nh:512 * nh + 512],
                                start=(c == 0), stop=(c == 5))
                        if with_bias:
                            nc.scalar.activation(
                                out=qk16[:, f, 512 * nh:512 * nh + 512], in_=ps[:],
                                func=AF.Identity, bias=bqkc[:, f:f + 1],
                                scale=SCALE if f < 6 else 1.0)
                        elif (f + nh) % 2 == 0:
                            nc.scalar.activation(
                                out=qk16[:, f, 512 * nh:512 * nh + 512], in_=ps[:],
                                func=AF.Copy, bias=0.0,
                                scale=SCALE if f < 6 else 1.0)
                        else:
                            nc.vector.tensor_scalar_mul(
                                out=qk16[:, f, 512 * nh:512 * nh + 512],
                                in0=ps[:], scalar1=SCALE if f < 6 else 1.0)
            # --- VG (token-major) + gelu + va ---
            for t in range(8):
                vg_t = ph2.tile([128, 2 * H], F16, tag="vg")
                for fc in range(3):
                    ps = ph2ps.tile([128, 512], F32, tag="ps2")
                    for c in range(6):
                        nc.tensor.matmul(
                            ps[:], hT[:, c, 128 * t:128 * t + 128],
                            wvgT[:, c, 512 * fc:512 * fc + 512],
                            start=(c == 0), stop=(c == 5))
                    if with_bias:
                        nc.vector.scalar_tensor_tensor(
                            out=vg_t[:, 512 * fc:512 * fc + 512], in0=ps[:], scalar=1.0,
                            in1=bvgr[:, 512 * fc:512 * fc + 512],
                            op0=OP.mult, op1=OP.add)
                    else:
                        nc.vector.tensor_copy(
                            out=vg_t[:, 512 * fc:512 * fc + 512], in_=ps[:])
                nc.scalar.activation(out=g16[:, t, :], in_=vg_t[:, H:2 * H], func=AF.Gelu)
                vav = va16[:, t, :].rearrange("p (h c) -> p h c", h=NH)
                nc.vector.tensor_scalar_mul(
                    out=vav[:, :, 0:64],
                    in0=vg_t[:, 0:H].rearrange("p (h c) -> p h c", h=NH),
                    scalar1=vmask16[:, t:t + 1])
                nc.vector.tensor_copy(
                    out=vav[:, :, 64],
                    in_=vmask16[:, t:t + 1].to_broadcast((128, NH)))

        # phases 1-2 done: free their SBUF/PSUM pools before attention
        ph2stack.close()

        # =================================================================
        # phase 3: attention per (b, head-pair), 1-deep software pipeline:
        # produce(i+1) [expansions+evictions+shears] is emitted BEFORE
        # consume(i) [scores+exp+PV] so the PE stream never waits on a shear.
        # =================================================================
        with tc.tile_pool(name="psA", bufs=2, space="PSUM") as psA, \
             tc.tile_pool(name="psSmall", bufs=1, space="PSUM") as psSmall, \
             tc.tile_pool(name="psSC", bufs=2, space="PSUM") as psSC, \
             tc.tile_pool(name="psT", bufs=1, space="PSUM") as psT, \
             tc.tile_pool(name="w4p", bufs=8) as w4p, \
             tc.tile_pool(name="sh4p", bufs=8) as sh4p, \
             tc.tile_pool(name="etp", bufs=10) as etp, \
             tc.tile_pool(name="tmpp", bufs=6) as tmpp:

            def produce(b, p):
                tok0 = 512 * b
                # w4[side][half] = [128, 4, WIN] f16
                w4 = [[w4p.tile([128, 4, WIN], F16, tag="w4",
                                name=f"w4_{side}_{half}")
                       for half in range(2)] for side in range(2)]
                for t in range(4):
                    ws = 384 - 128 * t
                    tok = tok0 + 128 * t
                    for side in range(2):   # 0: cq (q content), 1: ck (k content)
                        f = p if side == 0 else 6 + p
                        M = Mh if side == 0 else Mq
                        pas = []
                        for half in range(2):
                            r0 = 64 * half
                            pa = psA.tile([128, WIN], F32, tag="pa")
                            nc.tensor.matmul(
                                pa[:, 0:512], qk16[r0:r0 + 64, f, tok:tok + 128],
                                M[r0:r0 + 64, p, ws:ws + 512],
                                start=True, stop=True)
                            nc.tensor.matmul(
                                pa[:, 512:WIN], qk16[r0:r0 + 64, f, tok:tok + 128],
                                M[r0:r0 + 64, p, ws + 512:ws + 640],
                                start=True, stop=True)
                            pas.append(pa)
                        for half in range(2):
                            eng = nc.vector if half == 0 else nc.scalar
                            if half == 0:
                                nc.vector.tensor_copy(
                                    out=w4[side][half][:, t, 0:512],
                                    in_=pas[half][:, 0:512])
                                nc.scalar.activation(
                                    out=w4[side][half][:, t, 512:WIN],
                                    in_=pas[half][:, 512:WIN], func=AF.Copy)
                            else:
                                nc.scalar.activation(
                                    out=w4[side][half][:, t, 0:512],
                                    in_=pas[half][:, 0:512], func=AF.Copy)
                                nc.vector.tensor_copy(
                                    out=w4[side][half][:, t, 512:WIN],
                                    in_=pas[half][:, 512:WIN])
                # batched shear DMAs: cq on the sync HWDGE ring, ck on the
                # (otherwise idle) gpsimd SWDGE ring
                sh4 = [[sh4p.tile([128, 4, 512], F16, tag="sh4",
                                  name=f"sh4_{side}_{half}")
                        for half in range(2)] for side in range(2)]
                for half in range(2):
                    nc.sync.dma_start(out=sh4[0][half][:],
                                      in_=_shear4_ap(w4[0][half][:]))
                    nc.gpsimd.dma_start(out=sh4[1][half][:],
                                        in_=_shear4_ap(w4[1][half][:]))
                return (b, p, sh4)

            def consume(state):
                b, p, sh4 = state
                tok0 = 512 * b
                ctx_b = ctx16a if b == 0 else ctx16b
                et = [[None] * 4, [None] * 4]
                for u in range(4):
                    # f16 PE transposes of cq shear blocks into f16 PSUM
                    # (both heads packed into one bank: [128, 2, 512] f16)
                    ct = psT.tile([128, 2, 512], F16, tag="ct")
                    for half in range(2):
                        for t in range(4):
                            nc.tensor.matmul(
                                ct[:, half, 128 * t:128 * t + 128],
                                sh4[0][half][:, t, 128 * u:128 * u + 128],
                                ident16[:], is_transpose=True,
                                start=True, stop=True)
                    # CC matmuls, row-paired across the two heads
                    scs = []
                    for half in range(2):
                        r0 = 64 * half
                        sc = psSC.tile([128, 512], F32, tag="sc")
                        nc.tensor.matmul(
                            sc[:],
                            qk16[r0:r0 + 64, 6 + p, tok0 + 128 * u:tok0 + 128 * u + 128],
                            qk16[r0:r0 + 64, p, tok0:tok0 + 512],
                            start=True, stop=False)
                        scs.append(sc)
                    for half in range(2):
                        # tmp16 = cqT + cksh  (DVE), then PE identity-add
                        tmp16 = tmpp.tile([128, 512], F16, tag="tmp")
                        nc.vector.tensor_tensor(
                            out=tmp16[:], in0=ct[:, half, :],
                            in1=sh4[1][half][:, u, :], op=OP.add)
                        nc.tensor.matmul(scs[half][:], ident16[:], tmp16[:],
                                         start=False, stop=True)
                        e_u = etp.tile([128, 512], F16, tag="et")
                        nc.scalar.activation(out=e_u[:], in_=scs[half][:],
                                             func=AF.Exp, bias=negc_t[:],
                                             scale=1.0)
                        et[half][u] = e_u
                # -- PV + divide --
                for half in range(2):
                    hh = 2 * p + half
                    for t in range(4):
                        cps = psSmall.tile([128, 65], F32, tag="small")
                        for u in range(4):
                            nc.tensor.matmul(
                                cps[:], et[half][u][:, 128 * t:128 * t + 128],
                                va16[:, 4 * b + u, 65 * hh:65 * hh + 65],
                                start=(u == 0), stop=(u == 3))
                        rec = stats.tile([128, 1], F32, tag="rec")
                        nc.vector.reciprocal(out=rec[:], in_=cps[:, 64:65])
                        nc.vector.tensor_scalar_mul(
                            out=ctx_b[:, t, 64 * hh:64 * hh + 64],
                            in0=cps[:, 0:64], scalar1=rec[:])
                if p == 5:
                    # gate + LN2 in place, overlapped with the next batch's
                    # attention (DVE/ACT only; per-batch ctx tile so no
                    # cross-batch tile dependency)
                    for t in range(4):
                        nc.vector.tensor_mul(ctx_b[:, t, :], ctx_b[:, t, :],
                                             g16[:, 4 * b + t, :])
                        layernorm_to(ctx_b[:, t, :], ctx_b[:, t, :],
                                     f"ln2_{b}_{t}")

            pairs = [(b, p) for b in range(BL) for p in range(6)]
            pending = None
            for (b, p) in pairs:
                st = produce(b, p)
                if pending is not None:
                    consume(pending)
                pending = st
            consume(pending)

        # =================================================================
        # phase 4: on-chip transpose of LN2 output, out projection
        # =================================================================
        with tc.tile_pool(name="ph4ps", bufs=3, space="PSUM") as ph4ps, \
             tc.tile_pool(name="tps4", bufs=4, space="PSUM") as tps4, \
             tc.tile_pool(name="ph4", bufs=3) as ph4, \
             tc.tile_pool(name="ln2Tp", bufs=1) as ln2Tp:
            ln2T = ln2Tp.tile([128, 6, T], F16, tag="ln2T")
            for t in range(8):
                ctx_b = ctx16a if t < 4 else ctx16b
                for c in range(6):
                    tp = tps4.tile([128, 128], F16, tag="tp4")
                    nc.tensor.matmul(tp[:], ctx_b[:, t % 4, 128 * c:128 * c + 128],
                                     ident16[:], is_transpose=True,
                                     start=True, stop=True)
                    if c % 2 == 0:
                        nc.scalar.activation(
                            out=ln2T[:, c, 128 * t:128 * t + 128], in_=tp[:],
                            func=AF.Copy)
                    else:
                        nc.vector.tensor_copy(
                            out=ln2T[:, c, 128 * t:128 * t + 128], in_=tp[:])
            for t in range(8):
                ot = ph4.tile([128, H], F32, tag="ot")
                for fc, (f0, fw) in enumerate([(0, 512), (512, 256)]):
                    ps = ph4ps.tile([128, 512], F32, tag="ops")
                    for c in range(6):
                        nc.tensor.matmul(
                            ps[:, :fw], ln2T[:, c, 128 * t:128 * t + 128],
                            woutT[:, c, f0:f0 + fw],
                            start=(c == 0), stop=(c == 5))
                    if with_bias:
                        nc.vector.scalar_tensor_tensor(
                            out=ot[:, f0:f0 + fw], in0=ps[:, :fw], scalar=1.0,
                            in1=boutr[:, f0:f0 + fw], op0=OP.mult, op1=OP.add)
                    else:
                        nc.vector.tensor_copy(out=ot[:, f0:f0 + fw], in_=ps[:, :fw])
                nc.sync.dma_start(out=out_d[128 * t:128 * t + 128, :], in_=ot[:])

    return nc


# ---------------------------------------------------------------------------
# host side
# ---------------------------------------------------------------------------
def _host_prep(position_indices, attention_mask):
    pi = np.asarray(position_indices)
    gvec = np.empty(1023, np.int64)
    gvec[511:] = pi[:, 0]
    gvec[:512] = pi[0, ::-1]
    d = np.arange(S)[:, None] - np.arange(S)[None, :]
    assert np.array_equal(gvec[d + 511], pi), "position_indices not Toeplitz"
    e = np.arange(1023)
    E_cq = (np.arange(NB)[:, None] == gvec[1022 - e][None, :]).astype(np.float16)
    E_ck = (np.arange(NB)[:, None] == gvec[e][None, :]).astype(np.float16)
    E_cq = np.concatenate([E_cq, np.zeros((NB, 1), np.float16)], 1)
    E_ck = np.concatenate([E_ck, np.zeros((NB, 1), np.float16)], 1)
    am = np.asarray(attention_mask).reshape(B, S)
    vmask = (~am).astype(np.float32)
    return E_cq, E_ck, vmask


def kernel(hidden_states, relative_embedding, w_qk, b_qk, w_vg, b_vg,
           w_out, b_out, attention_mask, position_indices):
    from concourse.bass_utils import run_bass_kernel_spmd

    hidden_states = np.asarray(hidden_states, dtype=np.float32)
    relative_embedding = np.asarray(relative_embedding, dtype=np.float32)
    w_qk = np.asarray(w_qk, dtype=np.float32)
    w_vg = np.asarray(w_vg, dtype=np.float32)
    w_out = np.asarray(w_out, dtype=np.float32)
    b_qk = np.asarray(b_qk, dtype=np.float32)
    b_vg = np.asarray(b_vg, dtype=np.float32)
    b_out = np.asarray(b_out, dtype=np.float32)

    with_bias = bool(np.any(b_qk) or np.any(b_vg) or np.any(b_out))
    E_cq, E_ck, vmask = _host_prep(position_indices, attention_mask)

    nc = build_module(with_bias)
    common = dict(
        wqkT=np.ascontiguousarray(w_qk.T).astype(np.float16),
        wvgT=np.ascontiguousarray(w_vg.T).astype(np.float16),
        woutT=np.ascontiguousarray(w_out.T).astype(np.float16),
        relT=np.ascontiguousarray(relative_embedding.T).astype(np.float16),
        Ecq=E_cq, Eck=E_ck)
    if with_bias:
        sc_col = np.where(np.arange(12) < 6, SCALE, 1.0).astype(np.float32)
        common["bqkc"] = np.ascontiguousarray(
            b_qk.reshape(12, 128).T * sc_col[None, :])
        sc_row = np.concatenate([np.full(H, SCALE), np.ones(H)]).astype(np.float32)
        common["bqkr"] = (b_qk * sc_row)[None, :].astype(np.float32)
        common["bvgr"] = b_vg[None, :].astype(np.float32)
        common["boutr"] = b_out[None, :].astype(np.float32)

    in_maps = []
    for core in range(NCORES):
        bsel = [BL * core + i for i in range(BL)]
        hid = np.ascontiguousarray(
            hidden_states[:, bsel, :].transpose(1, 0, 2).reshape(T, H))
        vm = np.ascontiguousarray(vmask[bsel].reshape(T, 1))
        in_maps.append(dict(common, hid=hid, vmask=vm))

    res = run_bass_kernel_spmd(nc, in_maps, list(range(NCORES)))
    out = np.empty((S, B, H), np.float32)
    for core in range(NCORES):
        o = res.results[core]["out"].reshape(BL, S, H)
        for i in range(BL):
            out[:, BL * core + i, :] = o[i]
    return out


# revision 24
# speedup vs baseline: 1.2075x; 1.2075x over previous
"""Trainium2 Bass kernel for nn_Bert_44452911514066 (DeBERTa-style disentangled
attention BERT layer), data-parallel over batch across 8 NeuronCores.

kernel(**inputs) takes the FULL inputs (as produced by reference.setup_inputs)
and returns the FULL [S, B, H] output.

Key ideas (v2):
  - batch-DP: 2 batches per core, weights/tables replicated.
  - relative-position gather is Toeplitz: per (b,h) bucket values expand into
    diagonal space via matmuls (M matrices built from one-hot expansions),
    then sheared via single SBUF->SBUF DMAs with diagonal access patterns.
  - scores assembled transposed [k, q] in PSUM: CC matmul (+row-paired heads)
    + identity-matmul of (transposed cq + sheared ck), with the cq transpose
    done in f16 via PE transpose into an f16 PSUM tile.
  - softmax without max-subtraction: exp(s - 12); mask + denominator folded
    into an augmented/masked V matrix; division on DVE.
  - all matmuls f16 (full PE rate), fp32 accumulation.
  - on-chip PE transposes for h/ln2 (no DRAM round trip); expansion + CC
    matmuls of even/odd heads issued adjacently on disjoint PE row groups.
"""
import sys
sys.path.insert(0, "/opt/trn_rl_repo")
import math
import functools
import contextlib
import numpy as np

import concourse.bass as bass
import concourse.tile as tile
from concourse import mybir
from concourse.masks import make_identity

H, NH, HD, S, B = 768, 12, 64, 512, 16
NCORES = 8
BL = B // NCORES          # batches per core
T = BL * S                # tokens per core
SCALE = 1.0 / math.sqrt(3 * HD)
EPS = 1e-7
NB = 63                   # relative buckets
WIN = 640                 # expansion window per 128-row tile
CSHIFT = 12.0             # exp shift
F16 = mybir.dt.float16
F32 = mybir.dt.float32
AF = mybir.ActivationFunctionType
OP = mybir.AluOpType

# ---------------------------------------------------------------------------
# walrus workaround: this container's walrus accepts at most ONE sync wait per
# instruction; split extra waits onto single-wait NoOps.
# ---------------------------------------------------------------------------
from concourse.vector_clock import ScopedClock

_orig_add_instruction = tile.TileContext._add_instruction


def _patched_add_instruction(self, inst):
    si = inst.sync_info
    if si is not None and si.on_wait is not None and len(si.on_wait) > 1:
        waits = list(si.on_wait)
        for i, w in enumerate(waits[:-1]):
            nop = mybir.InstNoOp(name=f"{inst.name}-wsplit{i}", ins=[], outs=[])
            nop.engine = inst.engine
            nop.sync_info = mybir.SyncInfo(on_wait=[w], on_update=[])
            _orig_add_instruction(self, nop)
        inst.sync_info = mybir.SyncInfo(
            on_wait=[waits[-1]], on_update=list(si.on_update or []))
    _orig_add_instruction(self, inst)


def _patched_drain_and_barrier(self, tick_clock, wait_clock):
    nc = self.nc
    probe = nc.sync.nop(nofuse=True)
    wait_clock.add_sem_waits(probe.ins, ScopedClock({None: tick_clock.global_clock}))
    si = probe.ins.sync_info
    waits = list(si.on_wait) if si is not None and si.on_wait else []
    if len(waits) > 1:
        probe.ins.sync_info = mybir.SyncInfo(on_wait=waits[:1], on_update=[])
        for w in waits[1:]:
            n2 = nc.sync.nop(nofuse=True)
            n2.ins.sync_info = mybir.SyncInfo(on_wait=[w], on_update=[])
    nc.sync.drain()
    nc.all_engine_barrier()
    assert self.sems is not None
    popped = nc._tile_sem_poison_stack.pop()
    assert popped is self._sem_poison
    nc.clear_and_free_semaphores(list(self.sems.allocated().values()))
    nc.all_engine_barrier()


tile.TileContext._add_instruction = _patched_add_instruction
tile.TileContext._drain_and_barrier = _patched_drain_and_barrier


def _shear4_ap(t_ap):
    """Batched shear over 4 stacked windows.

    t_ap is a [128, 4, WIN] tile (partition p, window t, diag j).
    out[p, t, k] = tile_flat[p*(4*WIN-1) + t*WIN + 127 + k]
                 = tile[p, t, 127 + k - p]   for k in [0, 512).
    """
    return bass.AP(tensor=t_ap.tensor, offset=t_ap.offset + 127,
                   ap=[[4 * WIN - 1, 128], [WIN, 4], [1, 512]])


# ---------------------------------------------------------------------------
# device kernel build
# ---------------------------------------------------------------------------
@functools.lru_cache(maxsize=2)
def build_module(with_bias: bool):
    nc = bass.Bass()

    hid_d = nc.dram_tensor("hid", [T, H], F32, kind="ExternalInput")
    wqkT_d = nc.dram_tensor("wqkT", [H, 2 * H], F16, kind="ExternalInput")
    wvgT_d = nc.dram_tensor("wvgT", [H, 2 * H], F16, kind="ExternalInput")
    woutT_d = nc.dram_tensor("woutT", [H, H], F16, kind="ExternalInput")
    relT_d = nc.dram_tensor("relT", [H, NB], F16, kind="ExternalInput")
    Ecq_d = nc.dram_tensor("Ecq", [NB, 1024], F16, kind="ExternalInput")
    Eck_d = nc.dram_tensor("Eck", [NB, 1024], F16, kind="ExternalInput")
    vmask_d = nc.dram_tensor("vmask", [T, 1], F32, kind="ExternalInput")
    if with_bias:
        # host-prepared: bqkc[p, f] = b_qk[128f+p] * (SCALE if f<6 else 1)
        bqkc_d = nc.dram_tensor("bqkc", [128, 12], F32, kind="ExternalInput")
        # rows replicated for free-dim adds
        bqkr_d = nc.dram_tensor("bqkr", [1, 2 * H], F32, kind="ExternalInput")
        bvgr_d = nc.dram_tensor("bvgr", [1, 2 * H], F32, kind="ExternalInput")
        boutr_d = nc.dram_tensor("boutr", [1, H], F32, kind="ExternalInput")
    out_d = nc.dram_tensor("out", [T, H], F32, kind="ExternalOutput")

    with tile.TileContext(nc) as tc, contextlib.ExitStack() as ctx:
        persist = ctx.enter_context(tc.tile_pool(name="persist", bufs=1))
        stats = ctx.enter_context(tc.tile_pool(name="stats", bufs=4))
        # pools that live only through phases 1-2 (freed before attention)
        ph2stack = contextlib.ExitStack()
        ph2sb = ph2stack.enter_context(tc.tile_pool(name="ph2sb", bufs=1))

        # --- constants ---
        ident16 = persist.tile([128, 128], F16, tag="id16")
        make_identity(nc, ident16)
        eps_t = persist.tile([128, 1], F32, tag="eps")
        nc.vector.memset(eps_t, EPS)
        negc_t = persist.tile([128, 1], F32, tag="negc")
        nc.vector.memset(negc_t, -CSHIFT)

        # --- load weights / tables ---
        # hid tiles + vmask go first on the sync HWDGE ring (critical path for
        # LN1); all weight tables ride the scalar HWDGE ring.
        vmask16 = persist.tile([128, 8], F32, tag="vm")
        nc.sync.dma_start(
            out=vmask16[:],
            in_=vmask_d[:].rearrange("(t p) one -> p (t one)", p=128))
        wqkT = ph2sb.tile([128, 6, 2 * H], F16, tag="wqkT")
        wvgT = ph2sb.tile([128, 6, 2 * H], F16, tag="wvgT")
        woutT = persist.tile([128, 6, H], F16, tag="woutT")
        relT = ph2sb.tile([128, 6, NB], F16, tag="relT")
        for c in range(6):
            nc.scalar.dma_start(out=relT[:, c, :], in_=relT_d[128 * c:128 * c + 128, :])
        for c in range(6):
            nc.scalar.dma_start(out=wqkT[:, c, :], in_=wqkT_d[128 * c:128 * c + 128, :])
        Ecq = ph2sb.tile([NB, 1024], F16, tag="Ecq")
        Eck = ph2sb.tile([NB, 1024], F16, tag="Eck")
        nc.scalar.dma_start(out=Ecq[:], in_=Ecq_d[:])
        nc.scalar.dma_start(out=Eck[:], in_=Eck_d[:])
        for c in range(6):
            nc.scalar.dma_start(out=wvgT[:, c, :], in_=wvgT_d[128 * c:128 * c + 128, :])
        for c in range(6):
            nc.scalar.dma_start(out=woutT[:, c, :], in_=woutT_d[128 * c:128 * c + 128, :])
        if with_bias:
            bqkc = ph2sb.tile([128, 12], F32, tag="bqkc")
            nc.scalar.dma_start(out=bqkc[:], in_=bqkc_d[:])
            bqkr = ph2sb.tile([64, 2 * H], F32, tag="bqkr")
            nc.scalar.dma_start(
                out=bqkr[:],
                in_=bass.AP(tensor=bqkr_d, offset=0, ap=[[0, 64], [1, 2 * H]]))
            bvgr = ph2sb.tile([128, 2 * H], F32, tag="bvgr")
            nc.scalar.dma_start(
                out=bvgr[:],
                in_=bass.AP(tensor=bvgr_d, offset=0, ap=[[0, 128], [1, 2 * H]]))
            boutr = persist.tile([128, H], F32, tag="boutr")
            nc.scalar.dma_start(
                out=boutr[:],
                in_=bass.AP(tensor=boutr_d, offset=0, ap=[[0, 128], [1, H]]))

        def layernorm_to(out16, xin, tag):
            st = stats.tile([128, 3, 6], F32, tag="bnst")
            for sg in range(3):
                nc.vector.bn_stats(out=st[:, sg, :], in_=xin[:, 256 * sg:256 * sg + 256])
            mv = stats.tile([128, 2], F32, tag="bnmv")
            nc.vector.bn_aggr(out=mv[:], in_=st[:])
            rstd = stats.tile([128, 1], F32, tag="rstd")
            nc.scalar.activation(out=rstd[:], in_=mv[:, 1:2], func=AF.Sqrt,
                                 bias=eps_t[:], scale=1.0)
            nc.vector.reciprocal(out=rstd[:], in_=rstd[:])
            nc.vector.scalar_tensor_tensor(
                out=out16, in0=xin, scalar=mv[:, 0:1],
                in1=rstd[:].to_broadcast((128, H)),
                op0=OP.subtract, op1=OP.mult)

        # persistent activations
        hT = ph2sb.tile([128, 6, T], F16, tag="hT")
        qk16 = persist.tile([128, 12, T], F16, tag="qk16")
        g16 = persist.tile([128, 8, H], F16, tag="g16")
        va16 = persist.tile([128, 8, NH * 65], F16, tag="va16")
        ctx16a = persist.tile([128, 4, H], F16, tag="ctx16a")
        ctx16b = persist.tile([128, 4, H], F16, tag="ctx16b")
        posp = ph2sb.tile([64, 2 * H], F16, tag="posp")
        Mh = persist.tile([128, 6, 1024], F16, tag="Mh")
        Mq = persist.tile([128, 6, 1024], F16, tag="Mq")

        # =================================================================
        # phase 1+2: pos/M build, LN1 + on-chip transpose, QK/VG projections
        # =================================================================
        if True:
            ph2ps = ph2stack.enter_context(
                tc.tile_pool(name="ph2ps", bufs=4, space="PSUM"))
            tps = ph2stack.enter_context(
                tc.tile_pool(name="tps", bufs=4, space="PSUM"))
            ph2 = ph2stack.enter_context(tc.tile_pool(name="ph2", bufs=3))
            # --- LN1 + PE transpose into hT (first: hid DMAs land quickly) ---
            for t in range(8):
                xt = ph2.tile([128, H], F32, tag="x")
                nc.sync.dma_start(out=xt[:], in_=hid_d[128 * t:128 * t + 128, :])
                h16 = ph2.tile([128, H], F16, tag="h16")
                layernorm_to(h16[:], xt[:], f"ln1_{t}")
                for c in range(6):
                    tp = tps.tile([128, 128], F16, tag="tp")
                    nc.tensor.matmul(tp[:], h16[:, 128 * c:128 * c + 128],
                                     ident16[:], is_transpose=True,
                                     start=True, stop=True)
                    if c % 2 == 0:
                        nc.scalar.activation(
                            out=hT[:, c, 128 * t:128 * t + 128], in_=tp[:],
                            func=AF.Copy)
                    else:
                        nc.vector.tensor_copy(
                            out=hT[:, c, 128 * t:128 * t + 128], in_=tp[:])
            # --- pos projection ---
            for fc in range(3):
                ps = ph2ps.tile([128, 512], F32, tag="ps2")
                for c in range(6):
                    nc.tensor.matmul(
                        ps[:NB, :], relT[:, c, :], wqkT[:, c, 512 * fc:512 * fc + 512],
                        start=(c == 0), stop=(c == 5))
                if fc == 0:
                    segs = [(0, 512, SCALE)]
                elif fc == 1:
                    segs = [(0, 256, SCALE), (256, 512, 1.0)]
                else:
                    segs = [(0, 512, 1.0)]
                for (a, b_, sc) in segs:
                    if with_bias:
                        nc.vector.scalar_tensor_tensor(
                            out=posp[:NB, 512 * fc + a:512 * fc + b_],
                            in0=ps[:NB, a:b_], scalar=float(sc),
                            in1=bqkr[:NB, 512 * fc + a:512 * fc + b_],
                            op0=OP.mult, op1=OP.add)
                    else:
                        nc.vector.tensor_scalar_mul(
                            out=posp[:NB, 512 * fc + a:512 * fc + b_],
                            in0=ps[:NB, a:b_], scalar1=float(sc))
            # --- M matrices (per head pair; odd head in partitions 64-127;
            #     the two halves issued adjacently -> concurrent col groups) ---
            for p in range(6):
                for ec in range(2):
                    psq = ph2ps.tile([128, 512], F32, tag="ps2")
                    psk = ph2ps.tile([128, 512], F32, tag="ps2")
                    for half in range(2):
                        hh = 2 * p + half
                        r0 = 64 * half
                        nc.tensor.matmul(
                            psq[r0:r0 + 64, :],
                            posp[:NB, H + 64 * hh:H + 64 * hh + 64],
                            Ecq[:, 512 * ec:512 * ec + 512],
                            start=True, stop=True, tile_position=(0, r0))
                    for half in range(2):
                        hh = 2 * p + half
                        r0 = 64 * half
                        nc.tensor.matmul(
                            psk[r0:r0 + 64, :],
                            posp[:NB, 64 * hh:64 * hh + 64],
                            Eck[:, 512 * ec:512 * ec + 512],
                            start=True, stop=True, tile_position=(0, r0))
                    nc.scalar.activation(
                        out=Mh[:, p, 512 * ec:512 * ec + 512],
                        in_=psq[:], func=AF.Copy)
                    nc.vector.tensor_copy(
                        out=Mq[:, p, 512 * ec:512 * ec + 512],
                        in_=psk[:])

            # --- QK (feature-major; order so attention deps finish early) ---
            for p in range(6):
                for f in (p, 6 + p):
                    for nh in range(2):
                        ps = ph2ps.tile([128, 512], F32, tag="ps2")
                        for c in range(6):
                            nc.tensor.matmul(
                                ps[:], wqkT[:, c, 128 * f:128 * f + 128],
                                hT[:, c, 512 * nh:512 * nh + 512],
                                start=(c == 0), stop=(c == 5))
                        if with_bias:
                            nc.scalar.activation(
                                out=qk16[:, f, 512 * nh:512 * nh + 512], in_=ps[:],
                                func=AF.Identity, bias=bqkc[:, f:f + 1],
                                scale=SCALE if f < 6 else 1.0)
                        elif (f + nh) % 2 == 0:
                            nc.scalar.activation(
                                out=qk16[:, f, 512 * nh:512 * nh + 512], in_=ps[:],
                                func=AF.Copy, bias=0.0,
                                scale=SCALE if f < 6 else 1.0)
                        else:
                            nc.vector.tensor_scalar_mul(
                                out=qk16[:, f, 512 * nh:512 * nh + 512],
                                in0=ps[:], scalar1=SCALE if f < 6 else 1.0)
            # --- VG (token-major) + gelu + va ---
            for t in range(8):
                vg_t = ph2.tile([128, 2 * H], F16, tag="vg")
                for fc in range(3):
                    ps = ph2ps.tile([128, 512], F32, tag="ps2")
                    for c in range(6):
                        nc.tensor.matmul(
                            ps[:], hT[:, c, 128 * t:128 * t + 128],
                            wvgT[:, c, 512 * fc:512 * fc + 512],
                            start=(c == 0), stop=(c == 5))
                    if with_bias:
                        nc.vector.scalar_tensor_tensor(
                            out=vg_t[:, 512 * fc:512 * fc + 512], in0=ps[:], scalar=1.0,
                            in1=bvgr[:, 512 * fc:512 * fc + 512],
                            op0=OP.mult, op1=OP.add)
                    else:
                        nc.vector.tensor_copy(
                            out=vg_t[:, 512 * fc:512 * fc + 512], in_=ps[:])
                nc.scalar.activation(out=g16[:, t, :], in_=vg_t[:, H:2 * H], func=AF.Gelu)
                vav = va16[:, t, :].rearrange("p (h c) -> p h c", h=NH)
                nc.vector.tensor_scalar_mul(
                    out=vav[:, :, 0:64],
                    in0=vg_t[:, 0:H].rearrange("p (h c) -> p h c", h=NH),
                    scalar1=vmask16[:, t:t + 1])
                nc.vector.tensor_copy(
                    out=vav[:, :, 64],
                    in_=vmask16[:, t:t + 1].to_broadcast((128, NH)))

        # phases 1-2 done: free their SBUF/PSUM pools before attention
        ph2stack.close()

        # =================================================================
        # phase 3: attention per (b, head-pair), 1-deep software pipeline:
        # produce(i+1) [expansions+evictions+shears] is emitted BEFORE
        # consume(i) [scores+exp+PV] so the PE stream never waits on a shear.
        # =================================================================
        with tc.tile_pool(name="psA", bufs=2, space="PSUM") as psA, \
             tc.tile_pool(name="psSmall", bufs=1, space="PSUM") as psSmall, \
             tc.tile_pool(name="psSC", bufs=2, space="PSUM") as psSC, \
             tc.tile_pool(name="psT", bufs=1, space="PSUM") as psT, \
             tc.tile_pool(name="w4p", bufs=8) as w4p, \
             tc.tile_pool(name="sh4p", bufs=8) as sh4p, \
             tc.tile_pool(name="etp", bufs=10) as etp, \
             tc.tile_pool(name="tmpp", bufs=6) as tmpp:

            def produce(b, p):
                tok0 = 512 * b
                # w4[side][half] = [128, 4, WIN] f16
                w4 = [[w4p.tile([128, 4, WIN], F16, tag="w4",
                                name=f"w4_{side}_{half}")
                       for half in range(2)] for side in range(2)]
                for t in range(4):
                    ws = 384 - 128 * t
                    tok = tok0 + 128 * t
                    for side in range(2):   # 0: cq (q content), 1: ck (k content)
                        f = p if side == 0 else 6 + p
                        M = Mh if side == 0 else Mq
                        pas = []
                        for half in range(2):
                            r0 = 64 * half
                            pa = psA.tile([128, WIN], F32, tag="pa")
                            nc.tensor.matmul(
                                pa[:, 0:512], qk16[r0:r0 + 64, f, tok:tok + 128],
                                M[r0:r0 + 64, p, ws:ws + 512],
                                start=True, stop=True)
                            nc.tensor.matmul(
                                pa[:, 512:WIN], qk16[r0:r0 + 64, f, tok:tok + 128],
                                M[r0:r0 + 64, p, ws + 512:ws + 640],
                                start=True, stop=True)
                            pas.append(pa)
                        for half in range(2):
                            eng = nc.vector if half == 0 else nc.scalar
                            if half == 0:
                                nc.vector.tensor_copy(
                                    out=w4[side][half][:, t, 0:512],
                                    in_=pas[half][:, 0:512])
                                nc.scalar.activation(
                                    out=w4[side][half][:, t, 512:WIN],
                                    in_=pas[half][:, 512:WIN], func=AF.Copy)
                            else:
                                nc.scalar.activation(
                                    out=w4[side][half][:, t, 0:512],
                                    in_=pas[half][:, 0:512], func=AF.Copy)
                                nc.vector.tensor_copy(
                                    out=w4[side][half][:, t, 512:WIN],
                                    in_=pas[half][:, 512:WIN])
                # batched shear DMAs: cq on the sync HWDGE ring, ck on the
                # (otherwise idle) gpsimd SWDGE ring
                sh4 = [[sh4p.tile([128, 4, 512], F16, tag="sh4",
                                  name=f"sh4_{side}_{half}")
                        for half in range(2)] for side in range(2)]
                for half in range(2):
                    nc.sync.dma_start(out=sh4[0][half][:],
                                      in_=_shear4_ap(w4[0][half][:]))
                    nc.gpsimd.dma_start(out=sh4[1][half][:],
                                        in_=_shear4_ap(w4[1][half][:]))
                return (b, p, sh4)

            def consume(state):
                b, p, sh4 = state
                tok0 = 512 * b
                ctx_b = ctx16a if b == 0 else ctx16b
                et = [[None] * 4, [None] * 4]
                for u in range(4):
                    # f16 PE transposes of cq shear blocks into f16 PSUM
                    # (both heads packed into one bank: [128, 2, 512] f16)
                    ct = psT.tile([128, 2, 512], F16, tag="ct")
                    for half in range(2):
                        for t in range(4):
                            nc.tensor.matmul(
                                ct[:, half, 128 * t:128 * t + 128],
                                sh4[0][half][:, t, 128 * u:128 * u + 128],
                                ident16[:], is_transpose=True,
                                start=True, stop=True)
                    # CC matmuls, row-paired across the two heads
                    scs = []
                    for half in range(2):
                        r0 = 64 * half
                        sc = psSC.tile([128, 512], F32, tag="sc")
                        nc.tensor.matmul(
                            sc[:],
                            qk16[r0:r0 + 64, 6 + p, tok0 + 128 * u:tok0 + 128 * u + 128],
                            qk16[r0:r0 + 64, p, tok0:tok0 + 512],
                            start=True, stop=False)
                        scs.append(sc)
                    for half in range(2):
                        # tmp16 = cqT + cksh  (DVE), then PE identity-add
                        tmp16 = tmpp.tile([128, 512], F16, tag="tmp")
                        nc.vector.tensor_tensor(
                            out=tmp16[:], in0=ct[:, half, :],
                            in1=sh4[1][half][:, u, :], op=OP.add)
                        nc.tensor.matmul(scs[half][:], ident16[:], tmp16[:],
                                         start=False, stop=True)
                        e_u = etp.tile([128, 512], F16, tag="et")
                        nc.scalar.activation(out=e_u[:], in_=scs[half][:],
                                             func=AF.Exp, bias=negc_t[:],
                                             scale=1.0)
                        et[half][u] = e_u
                # -- PV + divide --
                for half in range(2):
                    hh = 2 * p + half
                    for t in range(4):
                        cps = psSmall.tile([128, 65], F32, tag="small")
                        for u in range(4):
                            nc.tensor.matmul(
                                cps[:], et[half][u][:, 128 * t:128 * t + 128],
                                va16[:, 4 * b + u, 65 * hh:65 * hh + 65],
                                start=(u == 0), stop=(u == 3))
                        rec = stats.tile([128, 1], F32, tag="rec")
                        nc.vector.reciprocal(out=rec[:], in_=cps[:, 64:65])
                        nc.vector.tensor_scalar_mul(
                            out=ctx_b[:, t, 64 * hh:64 * hh + 64],
                            in0=cps[:, 0:64], scalar1=rec[:])
                if p == 5:
                    # gate + LN2 in place, overlapped with the next batch's
                    # attention (DVE/ACT only; per-batch ctx tile so no
                    # cross-batch tile dependency)
                    for t in range(4):
                        nc.vector.tensor_mul(ctx_b[:, t, :], ctx_b[:, t, :],
                                             g16[:, 4 * b + t, :])
                        layernorm_to(ctx_b[:, t, :], ctx_b[:, t, :],
                                     f"ln2_{b}_{t}")

            pairs = [(b, p) for b in range(BL) for p in range(6)]
            pending = None
            for (b, p) in pairs:
                st = produce(b, p)
                if pending is not None:
                    consume(pending)
                pending = st
            consume(pending)

        # =================================================================
        # phase 4: on-chip transpose of LN2 output, out projection
        # =================================================================
        with tc.tile_pool(name="ph4ps", bufs=3, space="PSUM") as ph4ps, \
             tc.tile_pool(name="tps4", bufs=4, space="PSUM") as tps4, \
             tc.tile_pool(name="ph4", bufs=3) as ph4, \
             tc.tile_pool(name="ln2Tp", bufs=1) as ln2Tp:
            ln2T = ln2Tp.tile([128, 6, T], F16, tag="ln2T")
            for t in range(8):
                ctx_b = ctx16a if t < 4 else ctx16b
                for c in range(6):
                    tp = tps4.tile([128, 128], F16, tag="tp4")
                    nc.tensor.matmul(tp[:], ctx_b[:, t % 4, 128 * c:128 * c + 128],
                                     ident16[:], is_transpose=True,
                                     start=True, stop=True)
                    if c % 2 == 0:
                        nc.scalar.activation(
                            out=ln2T[:, c, 128 * t:128 * t + 128], in_=tp[:],
                            func=AF.Copy)
                    else:
                        nc.vector.tensor_copy(
                            out=ln2T[:, c, 128 * t:128 * t + 128], in_=tp[:])
            for t in range(8):
                ot = ph4.tile([128, H], F32, tag="ot")
                for fc, (f0, fw) in enumerate([(0, 512), (512, 256)]):
                    ps = ph4ps.tile([128, 512], F32, tag="ops")
                    for c in range(6):
                        nc.tensor.matmul(
                            ps[:, :fw], ln2T[:, c, 128 * t:128 * t + 128],
                            woutT[:, c, f0:f0 + fw],
                            start=(c == 0), stop=(c == 5))
                    if with_bias:
                        nc.vector.scalar_tensor_tensor(
                            out=ot[:, f0:f0 + fw], in0=ps[:, :fw], scalar=1.0,
                            in1=boutr[:, f0:f0 + fw], op0=OP.mult, op1=OP.add)
                    else:
                        nc.vector.tensor_copy(out=ot[:, f0:f0 + fw], in_=ps[:, :fw])
                nc.sync.dma_start(out=out_d[128 * t:128 * t + 128, :], in_=ot[:])

    return nc


# ---------------------------------------------------------------------------
# host side
# ---------------------------------------------------------------------------
def _host_prep(position_indices, attention_mask):
    pi = np.asarray(position_indices)
    gvec = np.empty(1023, np.int64)
    gvec[511:] = pi[:, 0]
    gvec[:512] = pi[0, ::-1]
    d = np.arange(S)[:, None] - np.arange(S)[None, :]
    assert np.array_equal(gvec[d + 511], pi), "position_indices not Toeplitz"
    e = np.arange(1023)
    E_cq = (np.arange(NB)[:, None] == gvec[1022 - e][None, :]).astype(np.float16)
    E_ck = (np.arange(NB)[:, None] == gvec[e][None, :]).astype(np.float16)
    E_cq = np.concatenate([E_cq, np.zeros((NB, 1), np.float16)], 1)
    E_ck = np.concatenate([E_ck, np.zeros((NB, 1), np.float16)], 1)
    am = np.asarray(attention_mask).reshape(B, S)
    vmask = (~am).astype(np.float32)
    return E_cq, E_ck, vmask


def kernel(hidden_states, relative_embedding, w_qk, b_qk, w_vg, b_vg,
           w_out, b_out, attention_mask, position_indices):
    from concourse.bass_utils import run_bass_kernel_spmd

    hidden_states = np.asarray(hidden_states, dtype=np.float32)
    relative_embedding = np.asarray(relative_embedding, dtype=np.float32)
    w_qk = np.asarray(w_qk, dtype=np.float32)
    w_vg = np.asarray(w_vg, dtype=np.float32)
    w_out = np.asarray(w_out, dtype=np.float32)
    b_qk = np.asarray(b_qk, dtype=np.float32)
    b_vg = np.asarray(b_vg, dtype=np.float32)
    b_out = np.asarray(b_out, dtype=np.float32)

    with_bias = bool(np.any(b_qk) or np.any(b_vg) or np.any(b_out))
    E_cq, E_ck, vmask = _host_prep(position_indices, attention_mask)

    nc = build_module(with_bias)
    common = dict(
        wqkT=np.ascontiguousarray(w_qk.T).astype(np.float16),
        wvgT=np.ascontiguousarray(w_vg.T).astype(np.float16),
        woutT=np.ascontiguousarray(w_out.T).astype(np.float16),
        relT=np.ascontiguousarray(relative_embedding.T).astype(np.float16),
        Ecq=E_cq, Eck=E_ck)
    if with_bias:
        sc_col = np.where(np.arange(12) < 6, SCALE, 1.0).astype(np.float32)
        common["bqkc"] = np.ascontiguousarray(
            b_qk.reshape(12, 128).T * sc_col[None, :])
        sc_row = np.concatenate([np.full(H, SCALE), np.ones(H)]).astype(np.float32)
        common["bqkr"] = (b_qk * sc_row)[None, :].astype(np.float32)
        common["bvgr"] = b_vg[None, :].astype(np.float32)
        common["boutr"] = b_out[None, :].astype(np.float32)

    in_maps = []
    for core in range(NCORES):
        bsel = [BL * core + i for i in range(BL)]
        hid = np.ascontiguousarray(
            hidden_states[:, bsel, :].transpose(1, 0, 2).reshape(T, H))
        vm = np.ascontiguousarray(vmask[bsel].reshape(T, 1))
        in_maps.append(dict(common, hid=hid, vmask=vm))

    res = run_bass_kernel_spmd(nc, in_maps, list(range(NCORES)))
    out = np.empty((S, B, H), np.float32)
    for core in range(NCORES):
        o = res.results[core]["out"].reshape(BL, S, H)
        for i in range(BL):
            out[:, BL * core + i, :] = o[i]
    return out
